# revision 12
# baseline (speedup 1.0000x reference)
"""Trainium2 Bass kernel for a 2-layer GraphConv + linear head (GCN-style).

Distribution: nodes (and their incident edges, by destination) are
partitioned across 8 NeuronCores. Weights are replicated. The per-layer
node-feature tables are exchanged with chunked AllGather collectives
(4 quarter-chunks per layer, so gathers on chunk q overlap the transfer
of chunk q+1).

Math (matches the reference):
    norm = clip(out_degree, 1)^-0.5           # per node, from src counts
    Y    = ((X * norm) @ w1)                  # layer1 matmul first (256>128)
    Z1   = segment_sum(Y[src] -> dst)
    H1   = relu(Z1 * norm + b1)
    G'   = (H1 * norm) @ w2                   # w2 hoisted (linear, commutes
                                              #  with segment_sum)
    Z2   = segment_sum(G'[src] -> dst)
    H2   = relu(Z2 * norm + b2)
    OUT  = H2 @ w3.T + b3
    return (OUT, OUT)

All node tables (Y, G') are fp16 and stored NODE-MAJOR (row = node,
256B contiguous), produced by operand-flipped matmuls (node block as the
stationary operand) so no transposed DMA is ever issued.  Features are
pre-normalized, pre-transposed and fp16-cast on the host.

The scatter-add (segment_sum) runs on the TensorEngine:
  * base pass: every (dst, chunk) gets SLOTS fixed gather slots; a window
    of 128 tokens covers 128/SLOTS dsts and is reduced with a constant
    block-diagonal ones matrix as the moving operand. Pad slots gather a
    zeroed table row.
  * overflow pass: edges beyond the fixed slots use data-driven one-hot
    windows (iota-vs-dstloc is_equal on the VectorEngine, fp16).
PSUM accumulates per-element (first matmul start=True clears the bank).

Gathers use the custom SWDGE dma_gather instruction (int16 indices); the
table is split into 4 quarter chunks (< 32768 rows each) that double as
the AllGather pipeline stages; each rank's contribution carries trailing
zero rows so every chunk contains a zero row for padding tokens.
"""

import numpy as np

import concourse.bass as bass
import concourse.bacc as bacc
import concourse.tile as tile
import concourse.mybir as mybir
from concourse import bass_utils

F32 = mybir.dt.float32
F16 = mybir.dt.float16
I16 = mybir.dt.int16

NC_CORES = 8
NCHUNK = 4
SLOTS = 4          # base gather slots per (dst, chunk)
WIN = 128          # tokens per scatter window (PE contraction dim)
WPD = WIN // SLOTS # dsts covered by one base window
PAD = 4            # zero pad rows per (core, chunk) contribution


class Plan:
    """Host-side preprocessing: slot/overflow assignment, index arrays,
    static (shared-across-cores) schedule."""

    def __init__(self, n_nodes, e_subgraph, tile_d=512):
        N = n_nodes
        assert N % NC_CORES == 0
        self.N = N
        self.NLOC = N // NC_CORES
        self.TILE_D = tile_d
        self.NT = -(-self.NLOC // tile_d)
        self.PADLOC = self.NT * tile_d

        # quarter chunks, tile-aligned
        qt = [round(self.NT * i / NCHUNK) for i in range(NCHUNK + 1)]
        assert qt[-1] == self.NT and all(qt[i] < qt[i + 1] for i in range(NCHUNK))
        self.qtile = qt
        self.qstart = [min(t * tile_d, self.NLOC) for t in qt]  # row bounds
        self.qlen = [self.qstart[i + 1] - self.qstart[i] for i in range(NCHUNK)]
        self.CONTRIB = [l + PAD for l in self.qlen]
        self.CHUNK = [NC_CORES * c for c in self.CONTRIB]
        assert all(c <= 32767 for c in self.CHUNK), self.CHUNK

        src = np.asarray(e_subgraph[0], dtype=np.int64)
        dst = np.asarray(e_subgraph[1], dtype=np.int64)

        deg = np.bincount(src, minlength=N).astype(np.float32)
        self.norm = np.clip(deg, 1.0, None) ** -0.5

        owner_s = src // self.NLOC
        lidx = src % self.NLOC
        qs = np.searchsorted(self.qstart, lidx, side="right") - 1
        qs = np.clip(qs, 0, NCHUNK - 1)
        qstart_arr = np.asarray(self.qstart[:NCHUNK], dtype=np.int64)
        contrib_arr = np.asarray(self.CONTRIB, dtype=np.int64)
        slidx = owner_s * contrib_arr[qs] + (lidx - qstart_arr[qs])
        schunk = qs
        owner = dst // self.NLOC
        dloc = dst % self.NLOC

        # zero row inside every chunk: core 0's first pad row
        self.zero_lidx = [self.qlen[c] for c in range(NCHUNK)]

        # per-core edge assignment
        per_core = []
        for c in range(NC_CORES):
            sel = owner == c
            dl, ch, li = dloc[sel], schunk[sel], slidx[sel]
            order = np.lexsort((ch, dl))
            dl, ch, li = dl[order], ch[order], li[order]
            key = dl * NCHUNK + ch
            is_new = np.r_[True, key[1:] != key[:-1]] if len(key) else np.array([], bool)
            grp_id = np.cumsum(is_new) - 1 if len(key) else key
            if len(key):
                grp_start = np.flatnonzero(is_new)
                rank = np.arange(len(key)) - grp_start[grp_id]
            else:
                rank = key
            per_core.append((dl, ch, li, rank))

        # base arrays + overflow lists
        NT, TILE_D = self.NT, self.TILE_D
        base = [np.full((NT, NCHUNK, TILE_D * SLOTS), -1, np.int64)
                for _ in range(NC_CORES)]
        for cc in range(NCHUNK):
            for c in range(NC_CORES):
                base[c][:, cc, :] = self.zero_lidx[cc]
        ovf = [[[([], []) for _ in range(NCHUNK)] for _ in range(NT)]
               for _ in range(NC_CORES)]
        for c in range(NC_CORES):
            dl, ch, li, rank = per_core[c]
            t = dl // TILE_D
            din = dl - t * TILE_D
            bm = rank < SLOTS
            base[c][t[bm], ch[bm], din[bm] * SLOTS + rank[bm]] = li[bm]
            om = ~bm
            for tt, cch, dd, ll in zip(t[om], ch[om], din[om], li[om]):
                ovf[c][tt][cch][0].append(ll)
                ovf[c][tt][cch][1].append(dd)

        # static overflow window counts (max over cores)
        self.nw = np.zeros((NT, NCHUNK), np.int64)
        for t in range(NT):
            for cc in range(NCHUNK):
                mx = max(len(ovf[c][t][cc][0]) for c in range(NC_CORES))
                self.nw[t, cc] = -(-mx // WIN) if mx else 0

        # token stream: per tile, per chunk: [base TILE_D*SLOTS][ovf nw*WIN]
        self.seg = np.zeros((NT, NCHUNK), np.int64)
        for t in range(NT):
            for cc in range(NCHUNK):
                self.seg[t, cc] = TILE_D * SLOTS + self.nw[t, cc] * WIN
        self.tile_tokens = self.seg.sum(axis=1)
        self.tile_groups = self.tile_tokens // WIN
        self.tot_cols = int(self.tile_tokens.sum()) // 16
        self.nw_tot = int(self.nw.sum())
        self.gmax_cc = [int(self.seg[:, cc].max()) // WIN for cc in range(NCHUNK)]

        # build per-core idx / dstloc arrays
        self.idx = np.zeros((NC_CORES, 128, self.tot_cols), np.int16)
        self.dstloc = np.full((NC_CORES, 128, max(self.nw_tot, 1)), -1.0,
                              np.float32)
        for c in range(NC_CORES):
            col = 0
            w_i = 0
            for t in range(NT):
                for cc in range(NCHUNK):
                    toks = np.full(int(self.seg[t, cc]), self.zero_lidx[cc],
                                   np.int64)
                    toks[:TILE_D * SLOTS] = base[c][t, cc]
                    ll, dd = ovf[c][t][cc]
                    if len(ll):
                        toks[TILE_D * SLOTS:TILE_D * SLOTS + len(ll)] = ll
                    seg = int(self.seg[t, cc])
                    wrapped = toks.astype(np.int16).reshape(seg // 16, 16).T
                    self.idx[c, :, col:col + seg // 16] = np.tile(wrapped, (8, 1))
                    col += seg // 16
                    for j in range(int(self.nw[t, cc])):
                        sl = dd[j * WIN:(j + 1) * WIN]
                        if len(sl):
                            self.dstloc[c, :len(sl), w_i] = \
                                np.asarray(sl, np.float32)
                        w_i += 1
            assert col == self.tot_cols

        # norm broadcast [128, PADLOC] per core (post-aggregation scale)
        self.normb = np.ones((NC_CORES, 128, self.PADLOC), np.float32)
        for c in range(NC_CORES):
            nl = self.norm[c * self.NLOC:(c + 1) * self.NLOC]
            self.normb[c, :, :self.NLOC] = nl[None, :]

    def consts(self):
        iota = np.broadcast_to(
            np.arange(self.TILE_D, dtype=np.float16), (128, self.TILE_D)).copy()
        onesb = np.zeros((128, WPD), np.float16)
        for tk in range(WIN):
            onesb[tk, tk // SLOTS] = 1.0
        return iota, onesb


def build_nc(plan: Plan, din, dh, dout):
    """Emit the bass program (shared SPMD across all cores)."""
    p = plan
    nc = bacc.Bacc("TRN2", target_bir_lowering=False, debug=False,
                   num_devices=NC_CORES, num_swdge_queues=4)

    kt = din // 128  # K-tiles for layer-1 matmul
    TD = p.TILE_D
    NB = TD // 128   # node blocks per tile

    featsT = nc.dram_tensor("featsT", [din, p.PADLOC], F16, kind="ExternalInput")
    w1_d = nc.dram_tensor("w1f", [128, kt, dh], F16, kind="ExternalInput")
    w2_d = nc.dram_tensor("w2f", [dh, dh], F16, kind="ExternalInput")
    w3t_d = nc.dram_tensor("w3tf", [dh, dout], F16, kind="ExternalInput")
    b1_d = nc.dram_tensor("b1", [dh, 1], F32, kind="ExternalInput")
    b2_d = nc.dram_tensor("b2", [dh, 1], F32, kind="ExternalInput")
    b3_d = nc.dram_tensor("b3", [dout, 1], F32, kind="ExternalInput")
    normb_d = nc.dram_tensor("normb", [128, p.PADLOC], F32, kind="ExternalInput")
    idx_d = nc.dram_tensor("idx", [128, p.tot_cols], I16, kind="ExternalInput")
    dstloc_d = nc.dram_tensor("dstloc", [128, max(p.nw_tot, 1)], F32,
                              kind="ExternalInput")
    iota_d = nc.dram_tensor("iota", [128, TD], F16, kind="ExternalInput")
    onesb_d = nc.dram_tensor("onesb", [128, WPD], F16, kind="ExternalInput")
    out_d = nc.dram_tensor("outT", [dout, p.PADLOC], F32, kind="ExternalOutput")

    y_loc = [nc.dram_tensor(f"y_loc{q}", [p.CONTRIB[q], dh], F16)
             for q in range(NCHUNK)]
    g_loc = [nc.dram_tensor(f"g_loc{q}", [p.CONTRIB[q], dh], F16)
             for q in range(NCHUNK)]
    t_y = [nc.dram_tensor(f"t_y{q}", [p.CHUNK[q], dh], F16, addr_space="Shared")
           for q in range(NCHUNK)]
    t_g = [nc.dram_tensor(f"t_g{q}", [p.CHUNK[q], dh], F16, addr_space="Shared")
           for q in range(NCHUNK)]


    def quarter_of_tile(t):
        for q in range(NCHUNK):
            if p.qtile[q] <= t < p.qtile[q + 1]:
                return q
        raise AssertionError(t)

    with tile.TileContext(nc) as tc:
        with (
            tc.tile_pool(name="const", bufs=1) as cp,
            tc.tile_pool(name="xt", bufs=3) as xtp,
            tc.tile_pool(name="yb", bufs=4) as ybp,
            tc.tile_pool(name="gath", bufs=2) as gp,
            tc.tile_pool(name="oh", bufs=4) as ohp,
            tc.tile_pool(name="mid", bufs=2) as midp,
            tc.tile_pool(name="psY", bufs=2, space="PSUM") as psY,
            tc.tile_pool(name="psB", bufs=2, space="PSUM") as psB,
            tc.tile_pool(name="psC", bufs=2, space="PSUM") as psC,
            tc.tile_pool(name="psD", bufs=2, space="PSUM") as psD,
        ):
            # ---- constants ----
            w1_sb = cp.tile([128, kt, dh], F16)
            nc.sync.dma_start(w1_sb[:], w1_d[:, :, :])
            w2_sb = cp.tile([dh, dh], F16)
            nc.sync.dma_start(w2_sb[:], w2_d[:, :])
            w3t_sb = cp.tile([dh, dout], F16)
            nc.sync.dma_start(w3t_sb[:], w3t_d[:, :])
            b1_sb = cp.tile([dh, 1], F32)
            nc.sync.dma_start(b1_sb[:], b1_d[:, :])
            b2_sb = cp.tile([dh, 1], F32)
            nc.sync.dma_start(b2_sb[:], b2_d[:, :])
            b3_sb = cp.tile([dout, 1], F32)
            nc.sync.dma_start(b3_sb[:], b3_d[:, :])
            iota_sb = cp.tile([128, TD], F16)
            nc.sync.dma_start(iota_sb[:], iota_d[:, :])
            onesb_sb = cp.tile([128, WPD], F16)
            nc.sync.dma_start(onesb_sb[:], onesb_d[:, :])
            dstloc_sb = cp.tile([128, max(p.nw_tot, 1)], F32)
            nc.sync.dma_start(dstloc_sb[:], dstloc_d[:, :])
            zeros_sb = cp.tile([128, dh], F16)
            nc.vector.memset(zeros_sb[:], 0.0)
            # whole-kernel resident: post-aggregation norm table + token idx
            normb_sb = cp.tile([128, p.PADLOC], F32)
            nc.scalar.dma_start(normb_sb[:], normb_d[:, :])
            idx_sb = cp.tile([128, p.tot_cols], I16)
            nc.scalar.dma_start(idx_sb[:], idx_d[:, :])

            # ---- phase A: Y = (X*norm) @ w1, node-major fp16, per quarter --
            for t in range(p.NT):
                q = quarter_of_tile(t)
                r0 = t * TD
                xt = xtp.tile([128, kt, TD], F16)
                src_ap = featsT[:, r0:r0 + TD].rearrange(
                    "(k x) n -> x k n", x=128)
                nc.scalar.dma_start(xt[:], src_ap)
                for nb4 in range(NB):
                    rows0 = r0 + nb4 * 128 - p.qstart[q]
                    nv = min(128, p.qlen[q] - rows0)
                    if nv <= 0:
                        break
                    ps = psY.tile([128, dh], F32, space="PSUM")
                    for k in range(kt):
                        nc.tensor.matmul(
                            ps[:], xt[:, k, nb4 * 128:(nb4 + 1) * 128],
                            w1_sb[:, k, :], start=(k == 0), stop=(k == kt - 1))
                    yb = ybp.tile([128, dh], F16)
                    nc.vector.tensor_copy(yb[:], ps[:])
                    nc.sync.dma_start(y_loc[q][rows0:rows0 + nv, :], yb[:nv, :])
                if t == p.qtile[q + 1] - 1:
                    nc.sync.dma_start(
                        y_loc[q][p.qlen[q]:p.CONTRIB[q], :], zeros_sb[:PAD, :])
                    nc.gpsimd.collective_compute(
                        "AllGather", mybir.AluOpType.bypass,
                        ins=[y_loc[q].ap().opt()], outs=[t_y[q].ap().opt()],
                        replica_groups=[list(range(NC_CORES))],
                    )

            # ---- aggregation layers ----
            def agg_layer(table, layer):
                col0 = 0
                w_i0 = 0
                for t in range(p.NT):
                    d0 = t * TD
                    tq = quarter_of_tile(t)
                    acc = psB.tile([128, TD], F32, space="PSUM")

                    # gathers (one per chunk, own tile each so matmuls on
                    # chunk cc start as soon as gather cc lands)
                    col = col0
                    g_cc = []
                    for cc in range(NCHUNK):
                        seg = int(p.seg[t, cc])
                        g_t = gp.tile([128, p.gmax_cc[cc], dh], F16,
                                      tag=f"g{cc}")
                        nc.gpsimd.dma_gather(
                            g_t[:, :seg // WIN, :],
                            table[cc][:, :],
                            idx_sb[:, col:col + seg // 16],
                            seg, seg, dh,
                            single_packet=False,
                            queue_num=(t * NCHUNK + cc) % 4,
                        )
                        g_cc.append(g_t)
                        col += seg // 16
                    col0 = col

                    spec = []  # (chunk, group, rhs_kind, info)
                    nbase = (TD * SLOTS) // WIN
                    for cc in range(NCHUNK):
                        for w in range(nbase):
                            spec.append((cc, w, "base", w))
                        for j in range(int(p.nw[t, cc])):
                            spec.append((cc, nbase + j, "ovf", None))

                    w_i = w_i0
                    for si, (cc, g, kind, info) in enumerate(spec):
                        start = si == 0
                        stop = si == len(spec) - 1
                        if kind == "base":
                            nc.tensor.matmul(
                                acc[:, info * WPD:(info + 1) * WPD],
                                g_cc[cc][:, g, :], onesb_sb[:],
                                start=start, stop=stop)
                        else:
                            oh = ohp.tile([128, TD], F16, tag="oh")
                            nc.vector.tensor_scalar(
                                out=oh[:], in0=iota_sb[:],
                                scalar1=dstloc_sb[:, w_i:w_i + 1], scalar2=None,
                                op0=mybir.AluOpType.is_equal)
                            nc.tensor.matmul(acc[:], g_cc[cc][:, g, :], oh[:],
                                             start=start, stop=stop)
                            w_i += 1
                    w_i0 = w_i

                    nb = normb_sb[:, d0:d0 + TD]

                    if layer == 1:
                        # H1 = relu(acc*norm + b1); G' = (H1*norm) @ w2
                        h = midp.tile([128, TD], F32, tag="h")
                        nc.vector.tensor_tensor(out=h[:], in0=acc[:], in1=nb,
                                                op=mybir.AluOpType.mult)
                        hr = midp.tile([128, TD], F32, tag="hr")
                        nc.scalar.activation(hr[:], h[:],
                                             mybir.ActivationFunctionType.Relu,
                                             bias=b1_sb[:, 0:1])
                        gt = midp.tile([128, TD], F16, tag="gt")
                        nc.vector.tensor_tensor(out=gt[:], in0=hr[:], in1=nb,
                                                op=mybir.AluOpType.mult)
                        for nb4 in range(NB):
                            rows0 = d0 + nb4 * 128 - p.qstart[tq]
                            nv = min(128, p.qlen[tq] - rows0)
                            if nv <= 0:
                                break
                            ps2 = psC.tile([128, dh], F32, space="PSUM")
                            nc.tensor.matmul(
                                ps2[:], gt[:, nb4 * 128:(nb4 + 1) * 128],
                                w2_sb[:], start=True, stop=True)
                            g2 = ybp.tile([128, dh], F16)
                            nc.vector.tensor_copy(g2[:], ps2[:])
                            nc.sync.dma_start(
                                g_loc[tq][rows0:rows0 + nv, :], g2[:nv, :])
                        if t == p.qtile[tq + 1] - 1:
                            nc.sync.dma_start(
                                g_loc[tq][p.qlen[tq]:p.CONTRIB[tq], :],
                                zeros_sb[:PAD, :])
                            nc.gpsimd.collective_compute(
                                "AllGather", mybir.AluOpType.bypass,
                                ins=[g_loc[tq].ap().opt()],
                                outs=[t_g[tq].ap().opt()],
                                replica_groups=[list(range(NC_CORES))],
                            )
                    else:
                        # H2 = relu(acc*norm + b2); OUT^T = w3t^T @ H2 + b3
                        h = midp.tile([128, TD], F32, tag="h")
                        nc.vector.tensor_tensor(out=h[:], in0=acc[:], in1=nb,
                                                op=mybir.AluOpType.mult)
                        h2 = midp.tile([128, TD], F16, tag="h2")
                        nc.scalar.activation(h2[:], h[:],
                                             mybir.ActivationFunctionType.Relu,
                                             bias=b2_sb[:, 0:1])
                        ps3 = psD.tile([dout, TD], F32, space="PSUM")
                        nc.tensor.matmul(ps3[:], w3t_sb[:], h2[:],
                                         start=True, stop=True)
                        ot = midp.tile([dout, TD], F32, tag="ot")
                        nc.vector.tensor_scalar(
                            out=ot[:], in0=ps3[:], scalar1=b3_sb[:, 0:1],
                            scalar2=None, op0=mybir.AluOpType.add)
                        nc.sync.dma_start(out_d[:, d0:d0 + TD], ot[:])

            agg_layer(t_y, layer=1)
            agg_layer(t_g, layer=2)

    nc.compile()
    return nc


def make_in_maps(plan: Plan, features, w1, b1, w2, b2, w3, b3):
    p = plan
    iota, onesb = p.consts()
    din = features.shape[1]
    dh = w1.shape[1]
    kt = din // 128
    w1f = np.ascontiguousarray(
        np.asarray(w1, np.float32).reshape(kt, 128, dh).transpose(1, 0, 2)
    ).astype(np.float16)
    in_maps = []
    for c in range(NC_CORES):
        xn = np.asarray(features[c * p.NLOC:(c + 1) * p.NLOC], np.float32) \
            * p.norm[c * p.NLOC:(c + 1) * p.NLOC][:, None]
        xt = np.zeros((din, p.PADLOC), np.float16)
        xt[:, :p.NLOC] = xn.T.astype(np.float16)
        in_maps.append(dict(
            featsT=xt,
            w1f=w1f,
            w2f=np.ascontiguousarray(w2, np.float16),
            w3tf=np.ascontiguousarray(np.asarray(w3).T, np.float16),
            b1=np.asarray(b1, np.float32).reshape(-1, 1),
            b2=np.asarray(b2, np.float32).reshape(-1, 1),
            b3=np.asarray(b3, np.float32).reshape(-1, 1),
            normb=p.normb[c],
            idx=p.idx[c],
            dstloc=p.dstloc[c],
            iota=iota,
            onesb=onesb,
        ))
    return in_maps


def assemble_output(plan: Plan, results, dout):
    p = plan
    h = np.empty((p.N, dout), np.float32)
    for c in range(NC_CORES):
        h[c * p.NLOC:(c + 1) * p.NLOC] = results[c]["outT"][:, :p.NLOC].T
    return h


def run_graphconv(n_nodes, e_subgraph, features, w1, b1, w2, b2, w3, b3,
                  tile_d=512, mode="hw", trace=False):
    plan = Plan(n_nodes, e_subgraph, tile_d=tile_d)
    nc = build_nc(plan, features.shape[1], w1.shape[1], w3.shape[0])
    in_maps = make_in_maps(plan, features, w1, b1, w2, b2, w3, b3)
    if mode == "sim":
        from concourse import bass_interp
        sim = bass_interp.MultiCoreSim(nc, num_cores=NC_CORES)
        for c in range(NC_CORES):
            for k, v in in_maps[c].items():
                sim.cores[c].tensor(k)[:] = v
        sim.simulate(check_with_hw=False)
        results = [{"outT": sim.cores[c].mem_tensor("outT")}
                   for c in range(NC_CORES)]
        res = None
    else:
        res = bass_utils.run_bass_kernel_spmd(
            nc, in_maps, list(range(NC_CORES)), trace=trace)
        results = res.results
    h = assemble_output(plan, results, w3.shape[0])
    return h, res


def kernel(n_subgraph, e_subgraph, to_fetch, features, w1, b1, w2, b2, w3, b3):
    h, _ = run_graphconv(
        n_subgraph.shape[0], e_subgraph, features, w1, b1, w2, b2, w3, b3)
    return (h, h)


# revision 17
# speedup vs baseline: 1.1045x; 1.1045x over previous
"""Trainium2 Bass kernel for a 2-layer GraphConv + linear head (GCN-style).

Distribution: nodes (and their incident edges, by destination) are
partitioned across 8 NeuronCores. Weights are replicated. The per-layer
node-feature tables are exchanged with chunked AllGather collectives
(4 quarter-chunks per layer, so gathers on chunk q overlap the transfer
of chunk q+1).

Math (matches the reference):
    norm = clip(out_degree, 1)^-0.5           # per node, from src counts
    Y    = ((X * norm) @ w1)                  # layer1 matmul first (256>128)
    Z1   = segment_sum(Y[src] -> dst)
    H1   = relu(Z1 * norm + b1)
    G'   = (H1 * norm) @ w2                   # w2 hoisted (linear, commutes
                                              #  with segment_sum)
    Z2   = segment_sum(G'[src] -> dst)
    H2   = relu(Z2 * norm + b2)
    OUT  = H2 @ w3.T + b3
    return (OUT, OUT)

All node tables (Y, G') are fp16 and stored NODE-MAJOR (row = node,
256B contiguous), produced by operand-flipped matmuls (node block as the
stationary operand) so no transposed DMA is ever issued.  Features are
pre-normalized, pre-transposed and fp16-cast on the host.

The scatter-add (segment_sum) runs on the TensorEngine:
  * base pass: every (dst, chunk) gets SLOTS fixed gather slots; a window
    of 128 tokens covers 128/SLOTS dsts and is reduced with a constant
    block-diagonal ones matrix as the moving operand. Pad slots gather a
    zeroed table row.
  * overflow pass: edges beyond the fixed slots use data-driven one-hot
    windows (iota-vs-dstloc is_equal on the VectorEngine, fp16).
PSUM accumulates per-element (first matmul start=True clears the bank).

Gathers use the custom SWDGE dma_gather instruction (int16 indices); the
table is split into 4 quarter chunks (< 32768 rows each) that double as
the AllGather pipeline stages; each rank's contribution carries trailing
zero rows so every chunk contains a zero row for padding tokens.
"""

import numpy as np

import concourse.bass as bass
import concourse.bacc as bacc
import concourse.tile as tile
import concourse.mybir as mybir
from concourse import bass_utils

F32 = mybir.dt.float32
F16 = mybir.dt.float16
I16 = mybir.dt.int16

NC_CORES = 8
NCHUNK = 4
SLOTS = 4          # base gather slots per (dst, chunk)
WIN = 128          # tokens per scatter window (PE contraction dim)
WPD = WIN // SLOTS # dsts covered by one base window
PAD = 4            # zero pad rows per (core, chunk) contribution


class Plan:
    """Host-side preprocessing: slot/overflow assignment, index arrays,
    static (shared-across-cores) schedule."""

    def __init__(self, n_nodes, e_subgraph, tile_d=512):
        N = n_nodes
        assert N % NC_CORES == 0
        self.N = N
        self.NLOC = N // NC_CORES
        self.TILE_D = tile_d
        self.NT = -(-self.NLOC // tile_d)
        self.PADLOC = self.NT * tile_d

        # quarter chunks, tile-aligned
        qt = [round(self.NT * i / NCHUNK) for i in range(NCHUNK + 1)]
        assert qt[-1] == self.NT and all(qt[i] < qt[i + 1] for i in range(NCHUNK))
        self.qtile = qt
        self.qstart = [min(t * tile_d, self.NLOC) for t in qt]  # row bounds
        self.qlen = [self.qstart[i + 1] - self.qstart[i] for i in range(NCHUNK)]
        self.CONTRIB = [l + PAD for l in self.qlen]
        self.CHUNK = [NC_CORES * c for c in self.CONTRIB]
        assert all(c <= 32767 for c in self.CHUNK), self.CHUNK

        src = np.asarray(e_subgraph[0], dtype=np.int64)
        dst = np.asarray(e_subgraph[1], dtype=np.int64)

        deg = np.bincount(src, minlength=N).astype(np.float32)
        self.norm = np.clip(deg, 1.0, None) ** -0.5

        owner_s = src // self.NLOC
        lidx = src % self.NLOC
        qs = np.searchsorted(self.qstart, lidx, side="right") - 1
        qs = np.clip(qs, 0, NCHUNK - 1)
        qstart_arr = np.asarray(self.qstart[:NCHUNK], dtype=np.int64)
        contrib_arr = np.asarray(self.CONTRIB, dtype=np.int64)
        slidx = owner_s * contrib_arr[qs] + (lidx - qstart_arr[qs])
        schunk = qs
        owner = dst // self.NLOC
        dloc = dst % self.NLOC

        # zero row inside every chunk: core 0's first pad row
        self.zero_lidx = [self.qlen[c] for c in range(NCHUNK)]

        # per-core edge assignment
        per_core = []
        for c in range(NC_CORES):
            sel = owner == c
            dl, ch, li = dloc[sel], schunk[sel], slidx[sel]
            order = np.lexsort((ch, dl))
            dl, ch, li = dl[order], ch[order], li[order]
            key = dl * NCHUNK + ch
            is_new = np.r_[True, key[1:] != key[:-1]] if len(key) else np.array([], bool)
            grp_id = np.cumsum(is_new) - 1 if len(key) else key
            if len(key):
                grp_start = np.flatnonzero(is_new)
                rank = np.arange(len(key)) - grp_start[grp_id]
            else:
                rank = key
            per_core.append((dl, ch, li, rank))

        # base arrays + overflow lists
        NT, TILE_D = self.NT, self.TILE_D
        base = [np.full((NT, NCHUNK, TILE_D * SLOTS), -1, np.int64)
                for _ in range(NC_CORES)]
        for cc in range(NCHUNK):
            for c in range(NC_CORES):
                base[c][:, cc, :] = self.zero_lidx[cc]
        ovf = [[[([], []) for _ in range(NCHUNK)] for _ in range(NT)]
               for _ in range(NC_CORES)]
        for c in range(NC_CORES):
            dl, ch, li, rank = per_core[c]
            t = dl // TILE_D
            din = dl - t * TILE_D
            bm = rank < SLOTS
            base[c][t[bm], ch[bm], din[bm] * SLOTS + rank[bm]] = li[bm]
            om = ~bm
            for tt, cch, dd, ll in zip(t[om], ch[om], din[om], li[om]):
                ovf[c][tt][cch][0].append(ll)
                ovf[c][tt][cch][1].append(dd)

        # static overflow window counts (max over cores)
        self.nw = np.zeros((NT, NCHUNK), np.int64)
        for t in range(NT):
            for cc in range(NCHUNK):
                mx = max(len(ovf[c][t][cc][0]) for c in range(NC_CORES))
                self.nw[t, cc] = -(-mx // WIN) if mx else 0

        # token stream: per tile, per chunk: [base TILE_D*SLOTS][ovf nw*WIN]
        self.seg = np.zeros((NT, NCHUNK), np.int64)
        for t in range(NT):
            for cc in range(NCHUNK):
                self.seg[t, cc] = TILE_D * SLOTS + self.nw[t, cc] * WIN
        self.tile_tokens = self.seg.sum(axis=1)
        self.tile_groups = self.tile_tokens // WIN
        self.tot_cols = int(self.tile_tokens.sum()) // 16
        self.nw_tot = int(self.nw.sum())
        self.gmax_cc = [int(self.seg[:, cc].max()) // WIN for cc in range(NCHUNK)]

        # build per-core idx / dstloc arrays
        self.idx = np.zeros((NC_CORES, 128, self.tot_cols), np.int16)
        self.dstloc = np.full((NC_CORES, 128, max(self.nw_tot, 1)), -1.0,
                              np.float32)
        for c in range(NC_CORES):
            col = 0
            w_i = 0
            for t in range(NT):
                for cc in range(NCHUNK):
                    toks = np.full(int(self.seg[t, cc]), self.zero_lidx[cc],
                                   np.int64)
                    toks[:TILE_D * SLOTS] = base[c][t, cc]
                    ll, dd = ovf[c][t][cc]
                    if len(ll):
                        toks[TILE_D * SLOTS:TILE_D * SLOTS + len(ll)] = ll
                    seg = int(self.seg[t, cc])
                    wrapped = toks.astype(np.int16).reshape(seg // 16, 16).T
                    self.idx[c, :, col:col + seg // 16] = np.tile(wrapped, (8, 1))
                    col += seg // 16
                    for j in range(int(self.nw[t, cc])):
                        sl = dd[j * WIN:(j + 1) * WIN]
                        if len(sl):
                            self.dstloc[c, :len(sl), w_i] = \
                                np.asarray(sl, np.float32)
                        w_i += 1
            assert col == self.tot_cols

        # norm broadcast [128, PADLOC] per core (post-aggregation scale)
        self.normb = np.ones((NC_CORES, 128, self.PADLOC), np.float32)
        for c in range(NC_CORES):
            nl = self.norm[c * self.NLOC:(c + 1) * self.NLOC]
            self.normb[c, :, :self.NLOC] = nl[None, :]

    def consts(self):
        iota = np.broadcast_to(
            np.arange(self.TILE_D, dtype=np.float16), (128, self.TILE_D)).copy()
        onesb = np.zeros((128, WPD), np.float16)
        for tk in range(WIN):
            onesb[tk, tk // SLOTS] = 1.0
        return iota, onesb


def build_nc(plan: Plan, din, dh, dout):
    """Emit the bass program (shared SPMD across all cores)."""
    p = plan
    nc = bacc.Bacc("TRN2", target_bir_lowering=False, debug=False,
                   num_devices=NC_CORES, num_swdge_queues=4)

    kt = din // 128  # K-tiles for layer-1 matmul
    TD = p.TILE_D
    NB = TD // 128   # node blocks per tile

    featsT = nc.dram_tensor("featsT", [din, p.PADLOC], F16, kind="ExternalInput")
    w1_d = nc.dram_tensor("w1f", [128, kt, dh], F16, kind="ExternalInput")
    w2_d = nc.dram_tensor("w2f", [dh, dh], F16, kind="ExternalInput")
    w3t_d = nc.dram_tensor("w3tf", [dh, dout], F16, kind="ExternalInput")
    b1_d = nc.dram_tensor("b1", [dh, 1], F32, kind="ExternalInput")
    b2_d = nc.dram_tensor("b2", [dh, 1], F32, kind="ExternalInput")
    b3_d = nc.dram_tensor("b3", [dout, 1], F32, kind="ExternalInput")
    normb_d = nc.dram_tensor("normb", [128, p.PADLOC], F32, kind="ExternalInput")
    idx_d = nc.dram_tensor("idx", [128, p.tot_cols], I16, kind="ExternalInput")
    dstloc_d = nc.dram_tensor("dstloc", [128, max(p.nw_tot, 1)], F32,
                              kind="ExternalInput")
    iota_d = nc.dram_tensor("iota", [128, TD], F16, kind="ExternalInput")
    onesb_d = nc.dram_tensor("onesb", [128, WPD], F16, kind="ExternalInput")
    out_d = nc.dram_tensor("outT", [dout, p.PADLOC], F32, kind="ExternalOutput")

    y_loc = [nc.dram_tensor(f"y_loc{q}", [p.CONTRIB[q], dh], F16)
             for q in range(NCHUNK)]
    g_loc = [nc.dram_tensor(f"g_loc{q}", [p.CONTRIB[q], dh], F16)
             for q in range(NCHUNK)]
    t_y = [nc.dram_tensor(f"t_y{q}", [p.CHUNK[q], dh], F16, addr_space="Shared")
           for q in range(NCHUNK)]
    t_g = [nc.dram_tensor(f"t_g{q}", [p.CHUNK[q], dh], F16, addr_space="Shared")
           for q in range(NCHUNK)]


    def quarter_of_tile(t):
        for q in range(NCHUNK):
            if p.qtile[q] <= t < p.qtile[q + 1]:
                return q
        raise AssertionError(t)

    with tile.TileContext(nc) as tc:
        with (
            tc.tile_pool(name="const", bufs=1) as cp,
            tc.tile_pool(name="xt", bufs=3) as xtp,
            tc.tile_pool(name="yb", bufs=4) as ybp,
            tc.tile_pool(name="nb", bufs=2) as nbp,
            tc.tile_pool(name="gath", bufs=2) as gp,
            tc.tile_pool(name="idxp", bufs=2) as ixp,
            tc.tile_pool(name="oh", bufs=4) as ohp,
            tc.tile_pool(name="mid", bufs=2) as midp,
            tc.tile_pool(name="psY", bufs=2, space="PSUM") as psY,
            tc.tile_pool(name="psB", bufs=2, space="PSUM") as psB,
            tc.tile_pool(name="psC", bufs=2, space="PSUM") as psC,
            tc.tile_pool(name="psD", bufs=2, space="PSUM") as psD,
        ):
            # ---- constants ----
            w1_sb = cp.tile([128, kt, dh], F16)
            nc.sync.dma_start(w1_sb[:], w1_d[:, :, :])
            w2_sb = cp.tile([dh, dh], F16)
            nc.sync.dma_start(w2_sb[:], w2_d[:, :])
            w3t_sb = cp.tile([dh, dout], F16)
            nc.sync.dma_start(w3t_sb[:], w3t_d[:, :])
            b1_sb = cp.tile([dh, 1], F32)
            nc.sync.dma_start(b1_sb[:], b1_d[:, :])
            b2_sb = cp.tile([dh, 1], F32)
            nc.sync.dma_start(b2_sb[:], b2_d[:, :])
            b3_sb = cp.tile([dout, 1], F32)
            nc.sync.dma_start(b3_sb[:], b3_d[:, :])
            iota_sb = cp.tile([128, TD], F16)
            nc.sync.dma_start(iota_sb[:], iota_d[:, :])
            onesb_sb = cp.tile([128, WPD], F16)
            nc.sync.dma_start(onesb_sb[:], onesb_d[:, :])
            dstloc_sb = cp.tile([128, max(p.nw_tot, 1)], F32)
            nc.sync.dma_start(dstloc_sb[:], dstloc_d[:, :])
            zeros_sb = cp.tile([128, dh], F16)
            nc.vector.memset(zeros_sb[:], 0.0)

            # ---- phase A: Y = (X*norm) @ w1, node-major fp16, per quarter --
            for t in range(p.NT):
                q = quarter_of_tile(t)
                r0 = t * TD
                xt = xtp.tile([128, kt, TD], F16)
                src_ap = featsT[:, r0:r0 + TD].rearrange(
                    "(k x) n -> x k n", x=128)
                nc.sync.dma_start(xt[:], src_ap)
                for nb4 in range(NB):
                    rows0 = r0 + nb4 * 128 - p.qstart[q]
                    nv = min(128, p.qlen[q] - rows0)
                    if nv <= 0:
                        break
                    ps = psY.tile([128, dh], F32, space="PSUM")
                    for k in range(kt):
                        nc.tensor.matmul(
                            ps[:], xt[:, k, nb4 * 128:(nb4 + 1) * 128],
                            w1_sb[:, k, :], start=(k == 0), stop=(k == kt - 1))
                    yb = ybp.tile([128, dh], F16)
                    nc.vector.tensor_copy(yb[:], ps[:])
                    nc.sync.dma_start(y_loc[q][rows0:rows0 + nv, :], yb[:nv, :])
                if t == p.qtile[q + 1] - 1:
                    nc.sync.dma_start(
                        y_loc[q][p.qlen[q]:p.CONTRIB[q], :], zeros_sb[:PAD, :])
                    nc.gpsimd.collective_compute(
                        "AllGather", mybir.AluOpType.bypass,
                        ins=[y_loc[q].ap().opt()], outs=[t_y[q].ap().opt()],
                        replica_groups=[list(range(NC_CORES))],
                    )

            # ---- aggregation layers ----
            def agg_layer(table, layer):
                col0 = 0
                w_i0 = 0
                for t in range(p.NT):
                    d0 = t * TD
                    tq = quarter_of_tile(t)
                    cols_t = int(p.tile_tokens[t]) // 16
                    g_t = gp.tile([128, int(p.tile_groups.max()), dh], F16,
                                  tag="gath")
                    ix = ixp.tile([128, int(p.tile_tokens.max()) // 16], I16,
                                  tag="idx")
                    nc.sync.dma_start(ix[:, :cols_t], idx_d[:, col0:col0 + cols_t])
                    acc = psB.tile([128, TD], F32, space="PSUM")

                    # gathers (one per chunk)
                    grp = 0
                    col = 0
                    spec = []  # (group, rhs_kind, info)
                    for cc in range(NCHUNK):
                        seg = int(p.seg[t, cc])
                        nc.gpsimd.dma_gather(
                            g_t[:, grp:grp + seg // WIN, :],
                            table[cc][:, :],
                            ix[:, col:col + seg // 16],
                            seg, seg, dh,
                            single_packet=False,
                            queue_num=(t * NCHUNK + cc) % 4,
                        )
                        nbase = (TD * SLOTS) // WIN
                        for w in range(nbase):
                            spec.append((grp + w, "base", w))
                        for j in range(int(p.nw[t, cc])):
                            spec.append((grp + nbase + j, "ovf", None))
                        grp += seg // WIN
                        col += seg // 16

                    w_i = w_i0
                    for si, (g, kind, info) in enumerate(spec):
                        start = si == 0
                        stop = si == len(spec) - 1
                        if kind == "base":
                            nc.tensor.matmul(
                                acc[:, info * WPD:(info + 1) * WPD],
                                g_t[:, g, :], onesb_sb[:],
                                start=start, stop=stop)
                        else:
                            oh = ohp.tile([128, TD], F16, tag="oh")
                            nc.vector.tensor_scalar(
                                out=oh[:], in0=iota_sb[:],
                                scalar1=dstloc_sb[:, w_i:w_i + 1], scalar2=None,
                                op0=mybir.AluOpType.is_equal)
                            nc.tensor.matmul(acc[:], g_t[:, g, :], oh[:],
                                             start=start, stop=stop)
                            w_i += 1
                    w_i0 = w_i
                    col0 += cols_t

                    nb = nbp.tile([128, TD], F32)
                    nc.sync.dma_start(nb[:], normb_d[:, d0:d0 + TD])

                    if layer == 1:
                        # H1 = relu(acc*norm + b1); G' = (H1*norm) @ w2
                        h = midp.tile([128, TD], F32, tag="h")
                        nc.vector.tensor_tensor(out=h[:], in0=acc[:], in1=nb[:],
                                                op=mybir.AluOpType.mult)
                        hr = midp.tile([128, TD], F32, tag="hr")
                        nc.scalar.activation(hr[:], h[:],
                                             mybir.ActivationFunctionType.Relu,
                                             bias=b1_sb[:, 0:1])
                        gt = midp.tile([128, TD], F16, tag="gt")
                        nc.vector.tensor_tensor(out=gt[:], in0=hr[:], in1=nb[:],
                                                op=mybir.AluOpType.mult)
                        for nb4 in range(NB):
                            rows0 = d0 + nb4 * 128 - p.qstart[tq]
                            nv = min(128, p.qlen[tq] - rows0)
                            if nv <= 0:
                                break
                            ps2 = psC.tile([128, dh], F32, space="PSUM")
                            nc.tensor.matmul(
                                ps2[:], gt[:, nb4 * 128:(nb4 + 1) * 128],
                                w2_sb[:], start=True, stop=True)
                            g2 = ybp.tile([128, dh], F16)
                            nc.vector.tensor_copy(g2[:], ps2[:])
                            nc.sync.dma_start(
                                g_loc[tq][rows0:rows0 + nv, :], g2[:nv, :])
                        if t == p.qtile[tq + 1] - 1:
                            nc.sync.dma_start(
                                g_loc[tq][p.qlen[tq]:p.CONTRIB[tq], :],
                                zeros_sb[:PAD, :])
                            nc.gpsimd.collective_compute(
                                "AllGather", mybir.AluOpType.bypass,
                                ins=[g_loc[tq].ap().opt()],
                                outs=[t_g[tq].ap().opt()],
                                replica_groups=[list(range(NC_CORES))],
                            )
                    else:
                        # H2 = relu(acc*norm + b2); OUT^T = w3t^T @ H2 + b3
                        h = midp.tile([128, TD], F32, tag="h")
                        nc.vector.tensor_tensor(out=h[:], in0=acc[:], in1=nb[:],
                                                op=mybir.AluOpType.mult)
                        h2 = midp.tile([128, TD], F16, tag="h2")
                        nc.scalar.activation(h2[:], h[:],
                                             mybir.ActivationFunctionType.Relu,
                                             bias=b2_sb[:, 0:1])
                        ps3 = psD.tile([dout, TD], F32, space="PSUM")
                        nc.tensor.matmul(ps3[:], w3t_sb[:], h2[:],
                                         start=True, stop=True)
                        ot = midp.tile([dout, TD], F32, tag="ot")
                        nc.vector.tensor_scalar(
                            out=ot[:], in0=ps3[:], scalar1=b3_sb[:, 0:1],
                            scalar2=None, op0=mybir.AluOpType.add)
                        nc.sync.dma_start(out_d[:, d0:d0 + TD], ot[:])

            agg_layer(t_y, layer=1)
            agg_layer(t_g, layer=2)

    nc.compile()
    return nc


def make_in_maps(plan: Plan, features, w1, b1, w2, b2, w3, b3):
    p = plan
    iota, onesb = p.consts()
    din = features.shape[1]
    dh = w1.shape[1]
    kt = din // 128
    w1f = np.ascontiguousarray(
        np.asarray(w1, np.float32).reshape(kt, 128, dh).transpose(1, 0, 2)
    ).astype(np.float16)
    in_maps = []
    for c in range(NC_CORES):
        xn = np.asarray(features[c * p.NLOC:(c + 1) * p.NLOC], np.float32) \
            * p.norm[c * p.NLOC:(c + 1) * p.NLOC][:, None]
        xt = np.zeros((din, p.PADLOC), np.float16)
        xt[:, :p.NLOC] = xn.T.astype(np.float16)
        in_maps.append(dict(
            featsT=xt,
            w1f=w1f,
            w2f=np.ascontiguousarray(w2, np.float16),
            w3tf=np.ascontiguousarray(np.asarray(w3).T, np.float16),
            b1=np.asarray(b1, np.float32).reshape(-1, 1),
            b2=np.asarray(b2, np.float32).reshape(-1, 1),
            b3=np.asarray(b3, np.float32).reshape(-1, 1),
            normb=p.normb[c],
            idx=p.idx[c],
            dstloc=p.dstloc[c],
            iota=iota,
            onesb=onesb,
        ))
    return in_maps


def assemble_output(plan: Plan, results, dout):
    p = plan
    h = np.empty((p.N, dout), np.float32)
    for c in range(NC_CORES):
        h[c * p.NLOC:(c + 1) * p.NLOC] = results[c]["outT"][:, :p.NLOC].T
    return h


def run_graphconv(n_nodes, e_subgraph, features, w1, b1, w2, b2, w3, b3,
                  tile_d=512, mode="hw", trace=False):
    plan = Plan(n_nodes, e_subgraph, tile_d=tile_d)
    nc = build_nc(plan, features.shape[1], w1.shape[1], w3.shape[0])
    in_maps = make_in_maps(plan, features, w1, b1, w2, b2, w3, b3)
    if mode == "sim":
        from concourse import bass_interp
        sim = bass_interp.MultiCoreSim(nc, num_cores=NC_CORES)
        for c in range(NC_CORES):
            for k, v in in_maps[c].items():
                sim.cores[c].tensor(k)[:] = v
        sim.simulate(check_with_hw=False)
        results = [{"outT": sim.cores[c].mem_tensor("outT")}
                   for c in range(NC_CORES)]
        res = None
    else:
        res = bass_utils.run_bass_kernel_spmd(
            nc, in_maps, list(range(NC_CORES)), trace=trace)
        results = res.results
    h = assemble_output(plan, results, w3.shape[0])
    return h, res


def kernel(n_subgraph, e_subgraph, to_fetch, features, w1, b1, w2, b2, w3, b3):
    h, _ = run_graphconv(
        n_subgraph.shape[0], e_subgraph, features, w1, b1, w2, b2, w3, b3)
    return (h, h)
